# revision 8
# baseline (speedup 1.0000x reference)
"""Trainium2 Bass kernel for nn_NodeConv (GNN message passing).

Strategy (8 NeuronCores, data-parallel, no collectives):
  - Nodes are partitioned into 8 contiguous ranges; every edge is routed to
    the core that owns its *destination* node, so the segment-sum is fully
    local to each core.  MLP weights are replicated.
  - Edge features are quantized to fp8 e4m3 with *error-feedback* along each
    node's edge list (the quantization residual of edge k is carried into
    edge k+1 before rounding), so the per-node segment-sum error is a single
    quantization step (~1e-2 end-to-end rel err) at HALF the fp16 traffic.
  - Nodes are degree-sorted and packed into groups of 128 / supergroups of
    512; edges are laid out feature-major and LEVEL-major: level k of a
    supergroup is the [128 feat x 512 nodes] slice holding each node's k-th
    edge.  The TensorEngine accumulates level PAIRS into PSUM with fp8
    DoubleRow matmuls against a stationary [I|I] identity (exact in fp8):
    512-wide streams that hide the per-matmul LDWEIGHTS cost.
  - The output-layer weights are pre-centered on the host
    (wo_c = Wo - rowsum(Wo)/C) so the MLP output is zero-mean per node and
    GroupNorm needs only a sum of squares; the normalize + residual is a
    single fused scalar_tensor_tensor per group writing fp16.
"""

import sys

sys.path.insert(0, "/opt/trn_rl_repo")

import numpy as np
import ml_dtypes

import concourse.bass as bass
import concourse.bacc as bacc
import concourse.tile as tile
from concourse import mybir
from concourse.bass_utils import run_bass_kernel_spmd

# bass_utils imports antenv.axon_hooks unconditionally when tracing is
# requested; the image's antenv lacks that module.  Provide a null registry
# so a BASS_TRACE env var can't crash the run.
try:
    import antenv.axon_hooks  # noqa: F401
except ImportError:
    import types as _types
    import antenv as _antenv
    _m = _types.ModuleType("antenv.axon_hooks")
    _m._hook = None
    _m.set_axon_ntff_profile_hook = lambda h, _m=_m: setattr(_m, "_hook", h)
    _m.get_axon_ntff_profile_hook = lambda _m=_m: _m._hook
    sys.modules["antenv.axon_hooks"] = _m
    _antenv.axon_hooks = _m

P = 128
N_CORES = 8
SG = 4          # groups per supergroup (MLP batch = 512 nodes)
EPS = 1e-5

F8 = mybir.dt.float8e4
F16 = mybir.dt.float16
F32 = mybir.dt.float32
AF = mybir.ActivationFunctionType
ALU = mybir.AluOpType
NP_F8 = ml_dtypes.float8_e4m3


# --------------------------------------------------------------------------
# Host-side sharding / layout
# --------------------------------------------------------------------------

def _host_prep(x, e, edge_index):
    """Shard nodes/edges across cores, fp8-feedback-quantize the edge
    features, and build per-core feature-major level-major ELL slabs."""
    n_nodes = x.shape[0]
    npc = -(-n_nodes // N_CORES)              # nodes per core (ceil)
    dst = np.asarray(edge_index[1]).astype(np.int64)
    e32 = np.ascontiguousarray(e, dtype=np.float32)
    n_edges = e32.shape[0]

    # fp8 edge store with a zero row for padding slots
    e_q = np.zeros((n_edges + 1, P), NP_F8)
    zero_row = n_edges

    cores = []
    for c in range(N_CORES):
        lo, hi = c * npc, min((c + 1) * npc, n_nodes)
        sel = np.nonzero((dst >= lo) & (dst < hi))[0]
        ldst = (dst[sel] - lo).astype(np.int64)
        n_real = hi - lo
        deg = np.bincount(ldst, minlength=npc)
        order = np.argsort(-deg, kind="stable")       # all npc local ids
        # edges sorted by local dst; esort[k] = global edge row
        order_e = np.argsort(ldst, kind="stable")
        esort = sel[order_e]
        ldst_s = ldst[order_e]
        starts = np.zeros(npc + 1, np.int64)
        np.cumsum(deg, out=starts[1:])
        # --- error-feedback quantization along each node's edge list ---
        pos = np.arange(len(esort)) - starts[ldst_s]
        carry = np.zeros((npc, P), np.float32)
        maxdeg = int(deg.max()) if len(esort) else 0
        for k in range(maxdeg):
            m = pos == k
            if not m.any():
                break
            rows = esort[m]
            seg = ldst_s[m]
            v = e32[rows] + carry[seg]
            q = v.astype(NP_F8)
            e_q[rows] = q
            carry[seg] = v - q.astype(np.float32)
        cores.append(dict(lo=lo, n_real=n_real, deg=deg, order=order,
                          esort=esort, starts=starts))

    # canonical group schedule, shared by all cores
    ngrp = -(-npc // P)
    ngrp = -(-ngrp // SG) * SG                # round up to supergroup multiple
    npc_pad = ngrp * P
    d_list = np.ones(ngrp, np.int64)
    for c in cores:
        degs = np.zeros(npc_pad, np.int64)
        degs[:npc] = c["deg"][c["order"]]
        dg = degs.reshape(ngrp, P).max(axis=1)
        d_list = np.maximum(d_list, dg)
    d_list = np.maximum(d_list, 1)
    nsg = ngrp // SG
    # per-supergroup level count (groups are degree-sorted so the 4 groups
    # of a supergroup have similar max degree -> little intra-sg padding)
    dsg = np.array([int(d_list[s * SG:(s + 1) * SG].max())
                    for s in range(nsg)], np.int64)
    w_sg = dsg * SG * P                        # slab cols per supergroup
    pairs = []                                 # (s0, n_sg)
    for s0 in range(0, nsg, 2):
        pairs.append((s0, min(2, nsg - s0)))
    w_pair = np.array([int(w_sg[s0:s0 + n].sum()) for s0, n in pairs],
                      np.int64)
    pair_offs = np.zeros(len(pairs) + 1, np.int64)
    np.cumsum(w_pair * P, out=pair_offs[1:])
    tot = int(pair_offs[-1])

    in_maps = []
    for c in cores:
        slab = np.zeros(tot, NP_F8)
        order = c["order"]
        deg, starts, esort = c["deg"], c["starts"], c["esort"]
        for pi, (s0, n) in enumerate(pairs):
            block_cols = []
            for s in range(s0, s0 + n):
                d = int(dsg[s])
                rows_all = np.empty((SG, P, d), np.int64)
                for gi in range(SG):
                    g = s * SG + gi
                    nid = order[g * P:(g + 1) * P]
                    nid_pad = np.zeros(P, np.int64)
                    degs_g = np.zeros(P, np.int64)
                    if len(nid):
                        nid_pad[:len(nid)] = nid
                        degs_g[:len(nid)] = deg[nid]
                    ks = np.arange(d)[None, :]
                    valid = ks < degs_g[:, None]
                    pos = starts[nid_pad][:, None] + ks
                    rows_all[gi] = np.where(
                        valid, esort[np.minimum(pos, len(esort) - 1)],
                        zero_row)
                arr = e_q[rows_all]                    # [SG, 128, d, 128f]
                # level-major: blk[f, k*512 + gi*128 + j] = arr[gi, j, k, f]
                blk = np.ascontiguousarray(
                    arr.transpose(3, 2, 0, 1)).reshape(P, d * SG * P)
                block_cols.append(blk)
            pair_block = np.concatenate(block_cols, axis=1)  # [128, W_pair]
            slab[pair_offs[pi]:pair_offs[pi + 1]] = pair_block.reshape(-1)
        in_maps.append(dict(e_ell=slab))

    meta = dict(npc=npc, ngrp=ngrp, npc_pad=npc_pad,
                d_list=d_list, dsg=dsg, pairs=pairs, pair_offs=pair_offs,
                w_pair=w_pair, w_sg=w_sg, tot=tot, cores=cores)
    return in_maps, meta


def _host_prep_x(x, meta):
    """Per-core x buffers: feature-major fp16 [128, npc_pad] (MLP input) and
    partition-major node-major fp16 [128, npc_pad] (residual; row p holds
    node p of each group, col g*128+ch)."""
    npc, npc_pad, ngrp = meta["npc"], meta["npc_pad"], meta["ngrp"]
    out = []
    for c in meta["cores"]:
        xp = np.zeros((npc_pad, P), np.float32)
        xr = np.asarray(x[c["lo"]:c["lo"] + c["n_real"]], np.float32)
        perm = c["order"]
        valid = perm < c["n_real"]
        xp[np.nonzero(valid)[0]] = xr[perm[valid]]
        xt = np.ascontiguousarray(xp.T.astype(np.float16))
        xpm = np.ascontiguousarray(
            xp.reshape(ngrp, P, P).transpose(1, 0, 2)
            .reshape(P, ngrp * P).astype(np.float16))
        out.append((xt, xpm))
    return out


# --------------------------------------------------------------------------
# Device program
# --------------------------------------------------------------------------

def _build_program(meta, flags):
    ngrp, npc_pad = meta["ngrp"], meta["npc_pad"]
    dsg = meta["dsg"]
    pairs, pair_offs = meta["pairs"], meta["pair_offs"]
    w_pair, tot = meta["w_pair"], meta["tot"]
    use_bo = flags["use_bo"]
    use_gn = flags["use_gn"]

    nc = bacc.Bacc("TRN2", target_bir_lowering=False, debug=False)

    e_ell = nc.dram_tensor("e_ell", [tot], F8, kind="ExternalInput").ap()
    xT_d = nc.dram_tensor("xT", [P, npc_pad], F16, kind="ExternalInput").ap()
    xpm_d = nc.dram_tensor("xpm", [P, npc_pad], F16, kind="ExternalInput").ap()
    w0x_d = nc.dram_tensor("W0x", [P, P], F16, kind="ExternalInput").ap()
    w0m_d = nc.dram_tensor("W0m", [P, P], F16, kind="ExternalInput").ap()
    wh0_d = nc.dram_tensor("Wh0", [P, P], F16, kind="ExternalInput").ap()
    wh1_d = nc.dram_tensor("Wh1", [P, P], F16, kind="ExternalInput").ap()
    woc_d = nc.dram_tensor("Woc", [P, P], F16, kind="ExternalInput").ap()
    b0_d = nc.dram_tensor("b0", [P, 1], F32, kind="ExternalInput").ap()
    bh0_d = nc.dram_tensor("bh0", [P, 1], F32, kind="ExternalInput").ap()
    bh1_d = nc.dram_tensor("bh1", [P, 1], F32, kind="ExternalInput").ap()
    ii8_d = nc.dram_tensor("II8", [P, 2 * P], F8, kind="ExternalInput").ap()
    if use_bo:
        bo_d = nc.dram_tensor("boc_b", [P, SG * P], F32,
                              kind="ExternalInput").ap()
    if use_gn:
        gnw_d = nc.dram_tensor("gnw_b", [P, SG * P], F32,
                               kind="ExternalInput").ap()
        gnb_d = nc.dram_tensor("gnb_b", [P, SG * P], F32,
                               kind="ExternalInput").ap()
    out_d = nc.dram_tensor("out", [P, npc_pad], F16, kind="ExternalOutput").ap()

    nsg = ngrp // SG
    W = SG * P

    with tile.TileContext(nc) as tc:
        with (
            tc.tile_pool(name="const", bufs=1) as cpool,
            tc.tile_pool(name="slab", bufs=3) as spool,
            tc.tile_pool(name="io", bufs=3) as iopool,
            tc.tile_pool(name="act", bufs=3) as apool,
            tc.tile_pool(name="gn", bufs=3) as gpool,
            tc.tile_pool(name="stat", bufs=4) as tpool,
            tc.tile_pool(name="pmsg", bufs=3, space="PSUM") as pmsg,
            tc.tile_pool(name="pmlp", bufs=2, space="PSUM") as pmlp,
            tc.tile_pool(name="pout", bufs=3, space="PSUM") as pout,
        ):
            ii8 = cpool.tile([P, 2 * P], F8)
            nc.sync.dma_start(ii8[:], ii8_d[:])
            eps_t = cpool.tile([P, 1], F32)
            nc.vector.memset(eps_t[:], EPS)
            w0x = cpool.tile([P, P], F16)
            nc.sync.dma_start(w0x[:], w0x_d[:])
            w0m = cpool.tile([P, P], F16)
            nc.sync.dma_start(w0m[:], w0m_d[:])
            wh0 = cpool.tile([P, P], F16)
            nc.sync.dma_start(wh0[:], wh0_d[:])
            wh1 = cpool.tile([P, P], F16)
            nc.sync.dma_start(wh1[:], wh1_d[:])
            woc = cpool.tile([P, P], F16)
            nc.sync.dma_start(woc[:], woc_d[:])
            b0 = cpool.tile([P, 1], F32)
            nc.sync.dma_start(b0[:], b0_d[:])
            bh0 = cpool.tile([P, 1], F32)
            nc.sync.dma_start(bh0[:], bh0_d[:])
            bh1 = cpool.tile([P, 1], F32)
            nc.sync.dma_start(bh1[:], bh1_d[:])
            if use_bo:
                bo_b = cpool.tile([P, W], F32)
                nc.sync.dma_start(bo_b[:], bo_d[:])
            if use_gn:
                gnw_b = cpool.tile([P, W], F32)
                nc.sync.dma_start(gnw_b[:], gnw_d[:])
                gnb_b = cpool.tile([P, W], F32)
                nc.sync.dma_start(gnb_b[:], gnb_d[:])

            ii8_v = ii8[:].rearrange("p (two n) -> p two n", two=2)

            # the whole output stays in SBUF (25.6 KB/partition fp16) and is
            # written back in ONE fat DMA at the end: per-pair output DMAs
            # produce small strided descriptors (~4x worse per byte) and
            # their buffer rotation back-pressures the compute pipeline.
            otp = cpool.tile([P, npc_pad], F16)

            # start the pipeline with the two smallest pairs (fast first
            # compute while the big slabs stream in); groups are sorted by
            # degree so the last pairs are the lightest.
            npair = len(pairs)
            if npair > 3:
                pair_order = ([npair - 1, npair - 2, npair - 3]
                              + list(range(npair - 3)))
            else:
                pair_order = list(range(npair))
            for pi in pair_order:
                s0, n_sg = pairs[pi]
                wp = int(w_pair[pi])
                wn = n_sg * W
                slab = spool.tile([P, wp], F8, tag="slab")
                nc.sync.dma_start(
                    slab[:],
                    e_ell[int(pair_offs[pi]):int(pair_offs[pi + 1])]
                    .rearrange("(p w) -> p w", p=P),
                )
                xtp = iopool.tile([P, wn], F16, tag="xT")
                nc.scalar.dma_start(xtp[:], xT_d[:, s0 * W:s0 * W + wn])
                xpm = iopool.tile([P, wn], F16, tag="xpm")
                nc.scalar.dma_start(xpm[:], xpm_d[:, s0 * W:s0 * W + wn])
                col = 0
                for si in range(n_sg):
                    s = s0 + si
                    # ---- message accumulate: msg[feat, node] via fp8
                    # DoubleRow level-pair matmuls against [I|I] ----
                    d = int(dsg[s])
                    msg_ps = pmsg.tile([P, W], F32, tag="msg")
                    k = 0
                    while k + 1 < d:
                        nc.tensor.matmul(
                            msg_ps[:],
                            lhsT=ii8_v,
                            rhs=slab[:, (col + k) * W:(col + k + 2) * W]
                            .rearrange("p (two n) -> p two n", two=2),
                            start=(k == 0),
                            stop=(k + 2 == d),
                            perf_mode=mybir.MatmulPerfMode.DoubleRow,
                        )
                        k += 2
                    if k < d:           # odd leftover level
                        nc.tensor.matmul(
                            msg_ps[:],
                            lhsT=ii8[:, 0:P],
                            rhs=slab[:, (col + k) * W:(col + k + 1) * W],
                            start=(k == 0),
                            stop=True,
                        )
                    col += d
                    msg_s = apool.tile([P, W], F16, tag="msg_s")
                    nc.scalar.copy(msg_s[:], msg_ps[:])

                    # ---- MLP (feature-major, fp16 in / f32 accum) ----
                    xt = xtp[:, si * W:(si + 1) * W]

                    h_ps = pmlp.tile([P, W], F32, tag="mlp")
                    nc.tensor.matmul(h_ps[:], lhsT=w0x[:], rhs=xt,
                                     start=True, stop=False)
                    nc.tensor.matmul(h_ps[:], lhsT=w0m[:], rhs=msg_s[:],
                                     start=False, stop=True)
                    h1 = apool.tile([P, W], F16, tag="h")
                    nc.scalar.activation(h1[:], h_ps[:], AF.Relu,
                                         bias=b0[:, 0:1])

                    h_ps2 = pmlp.tile([P, W], F32, tag="mlp")
                    nc.tensor.matmul(h_ps2[:], lhsT=wh0[:], rhs=h1[:],
                                     start=True, stop=True)
                    h2 = apool.tile([P, W], F16, tag="h")
                    nc.vector.tensor_scalar(h2[:], h_ps2[:], bh0[:, 0:1], 0.0,
                                            op0=ALU.add, op1=ALU.max)

                    h_ps3 = pmlp.tile([P, W], F32, tag="mlp")
                    nc.tensor.matmul(h_ps3[:], lhsT=wh1[:], rhs=h2[:],
                                     start=True, stop=True)
                    h3 = apool.tile([P, W], F16, tag="h")
                    nc.scalar.activation(h3[:], h_ps3[:], AF.Relu,
                                         bias=bh1[:, 0:1])

                    # ---- output layer, node-major o[node, ch]; wo_c is
                    # pre-centered so o is zero-mean over channels ----
                    o_ps = pout.tile([P, W], F32, tag="o")
                    for gi in range(SG):
                        nc.tensor.matmul(
                            o_ps[:, gi * P:(gi + 1) * P],
                            lhsT=h3[:, gi * P:(gi + 1) * P], rhs=woc[:],
                            start=True, stop=True,
                        )

                    # ---- GroupNorm(1, C) + residual ----
                    if use_bo:
                        ob = gpool.tile([P, W], F32, tag="basis")
                        nc.vector.tensor_add(ob[:], o_ps[:], bo_b[:])
                    else:
                        ob = o_ps
                    sqf = gpool.tile([P, W], F32, tag="sqf")
                    nc.scalar.activation(sqf[:], ob[:], AF.Square)
                    s2 = tpool.tile([P, SG], F32, tag="s2")
                    nc.vector.tensor_reduce(
                        s2[:], sqf[:].rearrange("p (g c) -> p g c", c=P),
                        axis=mybir.AxisListType.X, op=ALU.add)
                    sd = tpool.tile([P, SG], F32, tag="sd")
                    nc.scalar.activation(sd[:], s2[:], AF.Sqrt,
                                         bias=eps_t[:, 0:1], scale=1.0 / P)
                    rinv = tpool.tile([P, SG], F32, tag="rinv")
                    nc.vector.reciprocal(rinv[:], sd[:])

                    off = s * W          # global col offset in the mega-otp
                    loff = si * W        # pair-local offset in xpm
                    if use_gn:
                        tn = gpool.tile([P, W], F32, tag="tn")
                        for gi in range(SG):
                            nc.vector.tensor_scalar_mul(
                                tn[:, gi * P:(gi + 1) * P],
                                ob[:, gi * P:(gi + 1) * P],
                                rinv[:, gi:gi + 1])
                        nc.vector.tensor_tensor(tn[:], tn[:], gnw_b[:],
                                                op=ALU.mult)
                        nc.vector.tensor_tensor(tn[:], tn[:], gnb_b[:],
                                                op=ALU.add)
                        nc.vector.tensor_tensor(
                            otp[:, off:off + W], tn[:], xpm[:, loff:loff + W],
                            op=ALU.add)
                    else:
                        # otp = o*rinv + x, fused per group
                        for gi in range(SG):
                            nc.vector.scalar_tensor_tensor(
                                otp[:, off + gi * P:off + (gi + 1) * P],
                                ob[:, gi * P:(gi + 1) * P],
                                rinv[:, gi:gi + 1],
                                xpm[:, loff + gi * P:loff + (gi + 1) * P],
                                op0=ALU.mult, op1=ALU.add)
            nc.sync.dma_start(out_d[:], otp[:])

    return nc


# --------------------------------------------------------------------------
# Entry point
# --------------------------------------------------------------------------

def _run(inputs, trace=False):
    x = np.asarray(inputs["x"], np.float32)
    e = np.asarray(inputs["e"], np.float32)
    edge_index = np.asarray(inputs["edge_index"])
    W0 = np.asarray(inputs["W0"], np.float32)
    b0 = np.asarray(inputs["b0"], np.float32)
    Wh = np.asarray(inputs["Wh"], np.float32)
    bh = np.asarray(inputs["bh"], np.float32)
    Wo = np.asarray(inputs["Wo"], np.float32)
    bo = np.asarray(inputs["bo"], np.float32)
    gn_w = np.asarray(inputs["gn_w"], np.float32)
    gn_b = np.asarray(inputs["gn_b"], np.float32)

    import time as _time
    _t0 = _time.monotonic()
    in_maps, meta = _host_prep(x, e, edge_index)
    xbufs = _host_prep_x(x, meta)
    print(f"[kernel] host prep {_time.monotonic()-_t0:.1f}s", flush=True)

    flags = dict(use_bo=bool(np.any(bo != 0.0)),
                 use_gn=bool(np.any(gn_w != 1.0) or np.any(gn_b != 0.0)))

    # centered output weights: rows sum to ~0 so o = h3 @ Woc is zero-mean
    Woc = Wo - Wo.sum(axis=1, keepdims=True) / P
    consts = dict(
        W0x=np.ascontiguousarray(W0[:P], np.float16),
        W0m=np.ascontiguousarray(W0[P:], np.float16),
        Wh0=np.ascontiguousarray(Wh[0], np.float16),
        Wh1=np.ascontiguousarray(Wh[1], np.float16),
        Woc=np.ascontiguousarray(Woc, np.float16),
        b0=b0.reshape(P, 1).copy(),
        bh0=bh[0].reshape(P, 1).copy(),
        bh1=bh[1].reshape(P, 1).copy(),
        II8=np.tile(np.eye(P, dtype=NP_F8), (1, 2)),
    )
    if flags["use_bo"]:
        # with centered weights the effective bias is also centered
        boc = bo - bo.mean()
        consts["boc_b"] = np.tile(boc[None, :], (P, SG)).astype(np.float32)
    if flags["use_gn"]:
        consts["gnw_b"] = np.tile(gn_w[None, :], (P, SG)).astype(np.float32)
        consts["gnb_b"] = np.tile(gn_b[None, :], (P, SG)).astype(np.float32)

    _t0 = _time.monotonic()
    nc = _build_program(meta, flags)
    print(f"[kernel] build {_time.monotonic()-_t0:.1f}s", flush=True)
    _t0 = _time.monotonic()
    nc.compile()
    print(f"[kernel] bacc compile {_time.monotonic()-_t0:.1f}s", flush=True)
    _t0 = _time.monotonic()

    full_maps = []
    for c in range(N_CORES):
        m = dict(in_maps[c])
        m["xT"], m["xpm"] = xbufs[c]
        m.update(consts)
        full_maps.append(m)

    res = run_bass_kernel_spmd(nc, full_maps, list(range(N_CORES)),
                               trace=trace)
    print(f"[kernel] spmd run {_time.monotonic()-_t0:.1f}s", flush=True)

    n_nodes = x.shape[0]
    out = np.empty((n_nodes, P), np.float32)
    ngrp = meta["ngrp"]
    for c, cinfo in enumerate(meta["cores"]):
        oc = np.asarray(res.results[c]["out"]).astype(np.float32)
        on = oc.reshape(P, ngrp, P).transpose(1, 0, 2).reshape(ngrp * P, P)
        perm = cinfo["order"]
        valid = perm < cinfo["n_real"]
        rows = np.nonzero(valid)[0]
        out[cinfo["lo"] + perm[valid]] = on[rows]
    return out, res


def kernel(**inputs):
    out, _ = _run(inputs, trace=False)
    return out
